# revision 13
# baseline (speedup 1.0000x reference)
"""NeuralPDA (vq_codebook) Trainium2 kernel.

Strategy (8 NeuronCores, SPMD):
  - The L=128-step PDA recurrence (batch 32) is replicated on every core:
    it is latency-bound, tiny, and replicating it removes all collectives.
  - The dominant cost -- logits = h_t @ W_out + b_out, a (4096, 32000) fp32
    output (524 MB) -- is sharded over the vocab dim: each core owns a
    4000-wide slice of W_out / b_out / logits.  Vocab matmuls are
    interleaved into the recurrence as soon as each 128-sample M-tile of
    h_t completes, so PE/DVE/DMA stream behind the recurrence.
  - The stack stores codebook *indices* (value = qidx+1, 0 = empty/zero
    vector).  top @ W_s is realized as a 513-way one-hot matmul against a
    host-precomputed table CW = [0; codebook] @ W_s.  All recurrence state
    is kept transposed (dims on partitions, batch on free) so every matmul
    uses natively-laid-out weights as lhsT and no per-step transposes are
    needed.
  - EMA accumulators (new_m, new_n) are computed at the end from staged
    quantization indices + codes via one-hot matmuls.

Self-contained: hardcodes all shapes from the problem spec.
"""

import numpy as np
from contextlib import ExitStack

import concourse.bass as bass
import concourse.bacc as bacc
import concourse.tile as tile
from concourse import mybir
from concourse.bass_utils import run_bass_kernel_spmd

F32 = mybir.dt.float32
F32R = mybir.dt.float32r
I32 = mybir.dt.int32
U8 = mybir.dt.uint8
U32 = mybir.dt.uint32
OP = mybir.AluOpType
ACT = mybir.ActivationFunctionType

B, L, V, D, NT, CD = 32, 128, 32000, 256, 512, 128
DECAY = 0.99
NCORES = 8
VS = V // NCORES          # vocab shard = 4000
NTILE = 500               # logits N-tile (<=512 f32 psum bank)
NN = VS // NTILE          # 8 N-tiles per core
S = B * L                 # 4096 samples, sample index s = t*32 + b
DEPTH = L + 2             # 130
KC = D // 128             # 2 contraction chunks of 128
CWC = 5                   # 513-entry one-hot -> 5 chunks of 128
MT = S // 128             # 32 logits M-tiles

# fp32r (TF32-like fast fp32) for the big vocab matmuls; accuracy is
# validated against the reference in test.py.  Set False for full fp32.
LOGITS_F32R = True
SCORES_F32R = False


def _b(ap, reps):
    """Free-dim broadcast of a (P, 1) AP to (P, reps)."""
    return ap.to_broadcast([ap.shape[0], reps])


def build_program():
    nc = bacc.Bacc(None)

    def p_in(name, shape, dtype=F32):
        return nc.declare_dram_parameter(name, list(shape), dtype, isOutput=False)

    def p_out(name, shape, dtype=F32):
        return nc.declare_dram_parameter(name, list(shape), dtype, isOutput=True)

    et_d = p_in("et", (D, S))                 # emb[x] gathered+transposed (host)
    xmask_d = p_in("xmask", (B, L))
    wi_d = p_in("wi", (D, D))
    wh_d = p_in("wh", (D, D))
    wt_d = p_in("wt", (D, D))
    wn_d = p_in("wn", (D, D))
    wcode_d = p_in("wcode", (D, D))
    cw_d = p_in("cw", (128, CWC * D))         # [0;cb]@W_s packed (p, c*256+j)
    cbt_d = p_in("cbt", (CD, NT))             # codebook.T
    wout_d = p_in("wout", (128, KC * VS),
                  F32R if LOGITS_F32R else F32)  # W_out shard packed (p, k*4000+j)
    bout_d = p_in("bout", (128, VS))          # b_out shard replicated
    macc_d = p_in("macc", (128, 4 * CD))      # m_acc packed (p, c*128+d)
    nacc_d = p_in("nacc", (1, NT))
    iota130_d = p_in("iota130", (B, DEPTH))
    i640_d = p_in("i640", (128, CWC))         # i640[p,c] = p + 128c
    iota512_d = p_in("iota512", (128, NT))    # row 0..511 on every partition

    logits_d = p_out("logits", (S, VS))
    pushes_d = p_out("pushes", (2 * B, L), I32)
    rawc_d = p_out("rawc", (2 * S, CD))
    newm_d = p_out("newm", (4, 128, CD))
    newn_d = p_out("newn", (1, NT))

    sdt = F32R if SCORES_F32R else F32

    ctx = ExitStack()
    with tile.TileContext(nc) as tc:
        persist = ctx.enter_context(tc.tile_pool(name="persist", bufs=1))

        # ---- persistent SBUF tensors ------------------------------------
        ldt = F32R if LOGITS_F32R else F32
        # ht: written ONLY by the per-step tanh (f32r producer), read only by
        # the f32r vocab matmuls.  ewt: fp32 EWT = (e@W_i).T.
        ht = [persist.tile([128, S], ldt, tag=f"bufA{k}", name=f"bufA{k}") for k in range(KC)]
        ewt = [persist.tile([128, S], F32, tag=f"bufB{k}", name=f"bufB{k}") for k in range(KC)]
        codesT = persist.tile([128, 64 * L], F32)      # (cd, t*64 + k*32 + b)
        wh_s = persist.tile([128, KC, D], F32)
        wt_s = persist.tile([128, KC, D], F32)
        wn_s = persist.tile([128, KC, D], F32)
        wcode_s = persist.tile([128, KC, D], F32)
        cw_s = persist.tile([128, CWC, D], F32)
        cbt_s = persist.tile([CD, NT], F32)
        wout_s = persist.tile([128, KC, VS], ldt)
        bout_s = persist.tile([128, VS], F32)
        macc_s = persist.tile([128, 4, CD], F32)
        nacc_s = persist.tile([1, NT], F32)
        iota130_s = persist.tile([B, DEPTH], F32)
        i640_s = persist.tile([128, CWC], F32)
        iota512_s = persist.tile([128, NT], F32)
        xmask_s = persist.tile([B, L], F32)
        stack = persist.tile([B, DEPTH], F32)
        qe2 = persist.tile([128, 64], F32)             # EMA indices, chunk-major
        pushf = persist.tile([2 * B, L], F32)
        ones_row = persist.tile([1, 128], F32)
        ones_col = persist.tile([128, 1], F32)
        tcol = persist.tile([B, 32], F32)              # transpose staging

        for w_s, w_d in ((wh_s, wh_d), (wt_s, wt_d), (wn_s, wn_d), (wcode_s, wcode_d)):
            for k in range(KC):
                nc.sync.dma_start(out=w_s[:, k, :], in_=w_d[k * 128:(k + 1) * 128, :])
        nc.sync.dma_start(out=cw_s[:], in_=cw_d[:].rearrange("p (c j) -> p c j", c=CWC))
        nc.sync.dma_start(out=cbt_s[:], in_=cbt_d[:])
        nc.sync.dma_start(out=wout_s[:],
                          in_=wout_d[:].rearrange("p (k j) -> p k j", k=KC))
        nc.sync.dma_start(out=bout_s[:], in_=bout_d[:])
        nc.sync.dma_start(out=macc_s[:], in_=macc_d[:].rearrange("p (c j) -> p c j", c=4))
        nc.sync.dma_start(out=nacc_s[:], in_=nacc_d[:])
        nc.sync.dma_start(out=iota130_s[:], in_=iota130_d[:])
        nc.sync.dma_start(out=i640_s[:], in_=i640_d[:])
        nc.sync.dma_start(out=iota512_s[:], in_=iota512_d[:])
        nc.sync.dma_start(out=xmask_s[:], in_=xmask_d[:])

        nc.vector.memset(stack[:], 0.0)
        nc.vector.memset(tcol[:], 0.0)
        nc.vector.memset(ones_row[:], 1.0)
        nc.vector.memset(ones_col[:], 1.0)

        # ---- preamble: EWT = (e @ W_i).T = W_i.T @ E.T --------------------
        # E.T is staged one 128-row chunk at a time in a scratch tile that is
        # released before the loop (keeps ht[] exclusively tanh-written for
        # the fp32r producer rule, and stays under the SBUF cap).
        with tc.tile_pool(name="pre_sb", bufs=2) as pre_sb, \
             tc.tile_pool(name="pre_ps", bufs=2, space="PSUM") as pre_ps:
            wi_s = pre_sb.tile([128, KC, D], F32, tag="wi", bufs=1)
            for k in range(KC):
                nc.sync.dma_start(out=wi_s[:, k, :], in_=wi_d[k * 128:(k + 1) * 128, :])
            for k in range(KC):
                et_sc = pre_sb.tile([128, S], F32, tag="et_sc", bufs=1)
                nc.sync.dma_start(out=et_sc[:], in_=et_d[k * 128:(k + 1) * 128, :])
                for m in range(KC):
                    for s8 in range(S // 512):
                        acc = pre_ps.tile([128, 512], F32, tag="ewt_acc")
                        nc.tensor.matmul(
                            acc[:],
                            lhsT=wi_s[:, k, m * 128:(m + 1) * 128],
                            rhs=et_sc[:, s8 * 512:(s8 + 1) * 512],
                            start=True, stop=True)
                        dst = ewt[m][:, s8 * 512:(s8 + 1) * 512]
                        if k == 0:
                            nc.vector.tensor_copy(dst, acc[:])
                        else:
                            nc.vector.tensor_add(dst, dst, acc[:])

        # ---- recurrence + interleaved vocab matmuls ----------------------
        with tc.tile_pool(name="loop_sb", bufs=3) as loop_sb, \
             tc.tile_pool(name="tiny", bufs=4) as tiny, \
             tc.tile_pool(name="log_sb", bufs=3) as log_sb, \
             tc.tile_pool(name="ps_s", bufs=2, space="PSUM") as ps_s, \
             tc.tile_pool(name="ps_rep", bufs=1, space="PSUM") as ps_rep, \
             tc.tile_pool(name="ps_ht", bufs=2, space="PSUM") as ps_ht, \
             tc.tile_pool(name="ps_sc", bufs=1, space="PSUM") as ps_sc, \
             tc.tile_pool(name="ps_log", bufs=2, space="PSUM") as ps_log:

            ptr = tiny.tile([B, 1], F32, tag="ptr")
            nc.vector.memset(ptr[:], 1.0)
            hT = None  # h_0 = 0: skip W_h matmuls at t=0
            emitted_u = 0

            def emit_logit_unit(u):
                m, n = u // NN, u % NN
                pl = ps_log.tile([128, NTILE], F32, tag="pl")
                for k in range(KC):
                    nc.tensor.matmul(
                        pl[:],
                        lhsT=ht[k][:, m * 128:(m + 1) * 128],
                        rhs=wout_s[:, k, n * NTILE:(n + 1) * NTILE],
                        start=(k == 0), stop=(k == KC - 1))
                lsb = log_sb.tile([128, NTILE], F32, tag="lsb")
                nc.vector.tensor_add(lsb[:], pl[:],
                                     bout_s[:, n * NTILE:(n + 1) * NTILE])
                nc.sync.dma_start(
                    out=logits_d[m * 128:(m + 1) * 128, n * NTILE:(n + 1) * NTILE],
                    in_=lsb[:])

            for t in range(L):
                # ---------- pop ----------
                alive = tiny.tile([B, 1], F32, tag="alive")
                nc.vector.tensor_scalar(alive[:], ptr[:], 0.0, None, OP.is_gt)
                ptr2 = tiny.tile([B, 1], F32, tag="ptr2")
                nc.vector.tensor_sub(ptr2[:], ptr[:], alive[:])
                tsel = loop_sb.tile([B, DEPTH], F32, tag="tsel")
                topraw = tiny.tile([B, 1], F32, tag="topraw")
                nc.vector.scalar_tensor_tensor(
                    tsel[:], iota130_s[:], ptr2[:, 0:1], stack[:],
                    OP.is_equal, OP.mult, accum_out=topraw[:])
                ti = tiny.tile([B, 1], F32, tag="ti")
                nc.vector.tensor_mul(ti[:], topraw[:], alive[:])
                sm = tiny.tile([B, 1], F32, tag="sm")
                nc.vector.tensor_mul(sm[:], xmask_s[:, t:t + 1], alive[:])

                # ---------- one-hot of top index ----------
                nc.vector.tensor_copy(tcol[:, 0:1], ti[:])
                trow = loop_sb.tile([B, 32], F32, tag="trow")
                nc.vector.transpose(trow[:], tcol[:])
                rep = ps_rep.tile([128, 32], F32, tag="rep")
                nc.tensor.matmul(rep[:], lhsT=ones_row[:], rhs=trow[0:1, 0:32],
                                 start=True, stop=True)
                oh = loop_sb.tile([128, CWC, 32], F32, tag="oh")
                for c in range(CWC):
                    nc.vector.tensor_scalar(oh[:, c, :], rep[:], i640_s[:, c:c + 1],
                                            None, OP.is_equal)

                # ---------- s_pre = (e@Wi).T[pre] + Ws.T top.T + Wh.T h.T --
                ps_spre = ps_s.tile([128, 2 * 32], F32, tag="spre")
                for m in range(KC):
                    dst = ps_spre[:, m * 32:(m + 1) * 32]
                    first = True
                    if hT is not None:
                        for k in range(KC):
                            nc.tensor.matmul(
                                dst, lhsT=wh_s[:, k, m * 128:(m + 1) * 128],
                                rhs=hT[:, k, :], start=first, stop=False)
                            first = False
                    for c in range(CWC):
                        nc.tensor.matmul(
                            dst, lhsT=cw_s[:, c, m * 128:(m + 1) * 128],
                            rhs=oh[:, c, :], start=first, stop=(c == CWC - 1))
                        first = False
                # s = tanh(psum + EWT[:, t])  -> new hT
                hT_new = loop_sb.tile([128, KC, 32], F32, tag="hT")
                spre_sb = loop_sb.tile([128, KC, 32], F32, tag="spre_sb")
                for m in range(KC):
                    nc.vector.tensor_add(spre_sb[:, m, :],
                                         ps_spre[:, m * 32:(m + 1) * 32],
                                         ewt[m][:, t * 32:(t + 1) * 32])
                    nc.scalar.activation(hT_new[:, m, :], spre_sb[:, m, :], ACT.Tanh)
                hT = hT_new

                # ---------- h_t, h_nt, codes ----------
                ps_proj = ps_ht.tile([128, 192], F32, tag="proj")
                for m in range(KC):
                    for k in range(KC):
                        nc.tensor.matmul(
                            ps_proj[:, m * 32:(m + 1) * 32],
                            lhsT=wt_s[:, k, m * 128:(m + 1) * 128],
                            rhs=hT[:, k, :], start=(k == 0), stop=(k == KC - 1))
                    for k in range(KC):
                        nc.tensor.matmul(
                            ps_proj[:, 64 + m * 32:96 + m * 32],
                            lhsT=wn_s[:, k, m * 128:(m + 1) * 128],
                            rhs=hT[:, k, :], start=(k == 0), stop=(k == KC - 1))
                hntT = loop_sb.tile([128, KC, 32], F32, tag="hntT")
                for m in range(KC):
                    nc.scalar.activation(ht[m][:, t * 32:(t + 1) * 32],
                                         ps_proj[:, m * 32:(m + 1) * 32], ACT.Tanh)
                    nc.scalar.activation(hntT[:, m, :],
                                         ps_proj[:, 64 + m * 32:96 + m * 32],
                                         ACT.Tanh)
                for m in range(KC):
                    for k in range(KC):
                        nc.tensor.matmul(
                            ps_proj[:, 128 + m * 32:160 + m * 32],
                            lhsT=wcode_s[:, k, m * 128:(m + 1) * 128],
                            rhs=hntT[:, k, :], start=(k == 0), stop=(k == KC - 1))
                cslice = codesT[:, t * 64:(t + 1) * 64]
                nc.vector.tensor_copy(cslice, ps_proj[:, 128:192])

                # ---------- VQ: scores + argmax ----------
                ps_score = ps_sc.tile([64, NT], F32, tag="score")
                nc.tensor.matmul(ps_score[:], lhsT=cslice.bitcast(sdt),
                                 rhs=cbt_s[:].bitcast(sdt), start=True, stop=True)
                mx = tiny.tile([64, 8], F32, tag="mx")
                nc.vector.max(mx[:], ps_score[:])
                mi = tiny.tile([64, 8], U32, tag="mi")
                nc.vector.max_index(mi[:], mx[:], ps_score[:])
                qf = tiny.tile([64, 1], F32, tag="qf")
                nc.vector.tensor_copy(qf[:], mi[:, 0:1])

                # ---------- staging: pushes + EMA indices ----------
                # replicate sm to 64 partitions so all multi-input ops share
                # base partition 0 (BIR constraint).
                sm64 = tiny.tile([2 * B, 1], F32, tag="sm64")
                nc.vector.tensor_copy(sm64[0:B, :], sm[:])
                nc.vector.tensor_copy(sm64[B:2 * B, :], sm[:])
                smm64 = tiny.tile([2 * B, 1], F32, tag="smm64")
                nc.vector.tensor_scalar(smm64[:], sm64[:], 1.0, None, OP.subtract)
                qrow = 64 * (t % 2)
                qcol = t // 2
                nc.vector.tensor_scalar(pushf[:, t:t + 1], qf[:],
                                        sm64[:, 0:1], None, OP.mult)
                # qe = qidx*sm + (sm-1)   (= qidx if live else -1)
                nc.vector.scalar_tensor_tensor(
                    qe2[qrow:qrow + 2 * B, qcol:qcol + 1],
                    qf[:], sm64[:, 0:1], smm64[:], OP.mult, OP.add)

                # ---------- stack pushes ----------
                pcur = ptr2
                for k in range(2):
                    mk = tiny.tile([B, 1], F32, tag=f"mk{k}")
                    nc.vector.tensor_scalar(mk[:], pushf[k * B:(k + 1) * B, t:t + 1],
                                            0.0, None, OP.not_equal)
                    wk = loop_sb.tile([B, DEPTH], U8, tag=f"wk{k}")
                    nc.vector.tensor_scalar(wk[:], iota130_s[:], pcur[:, 0:1],
                                            mk[:, 0:1], OP.is_equal, OP.mult)
                    vk = tiny.tile([B, 1], F32, tag=f"vk{k}")
                    nc.vector.tensor_scalar(vk[:], qf[k * B:(k + 1) * B, 0:1],
                                            1.0, None, OP.add)
                    nc.vector.copy_predicated(stack[:], wk[:], _b(vk[:, 0:1], DEPTH))
                    pnew = tiny.tile([B, 1], F32, tag=f"pnew{k}")
                    nc.vector.tensor_add(pnew[:], pcur[:], mk[:])
                    pcur = pnew
                ptr = pcur

                # ---------- interleaved vocab matmuls (paced 2/step) -------
                target = min(NN * MT * (t + 1) // L, NN * ((t + 1) // 4))
                while emitted_u < target:
                    emit_logit_unit(emitted_u)
                    emitted_u += 1

            while emitted_u < NN * MT:
                emit_logit_unit(emitted_u)
                emitted_u += 1

            # ---------- pushes output ----------
            pushi = log_sb.tile([2 * B, L], I32, tag="pushi")
            nc.vector.tensor_copy(pushi[:], pushf[:])
            nc.sync.dma_start(out=pushes_d[:], in_=pushi[:])

        # ---------- EMA + raw codes ----------
        with tc.tile_pool(name="ema_sb", bufs=3) as ema_sb, \
             tc.tile_pool(name="ema_ps", bufs=2, space="PSUM") as ema_ps, \
             tc.tile_pool(name="ema_acc", bufs=1, space="PSUM") as ema_acc:
            # identity matrix for PE transposes
            identity = ema_sb.tile([128, 128], F32, tag="ident")
            iot = ema_sb.tile([128, 1], I32, tag="iot")
            nc.gpsimd.iota(iot[:], pattern=[[0, 1]], base=0, channel_multiplier=1)
            iotf = ema_sb.tile([128, 1], F32, tag="iotf")
            nc.vector.tensor_copy(iotf[:], iot[:])
            nc.vector.tensor_scalar(identity[:], iota512_s[:, 0:128],
                                    iotf[:, 0:1], None, OP.is_equal)

            am = [ema_acc.tile([128, CD], F32, tag=f"am{c}", name=f"am{c}") for c in range(4)]
            an = ema_acc.tile([1, NT], F32, tag="an")
            for ch in range(64):
                oh_e = ema_sb.tile([128, NT], F32, tag="ohe")
                nc.gpsimd.tensor_scalar(oh_e[:], iota512_s[:],
                                        qe2[:, ch:ch + 1], None, OP.is_equal)
                pst = ema_ps.tile([128, 128], F32, tag="pst")
                nc.tensor.transpose(pst[:], codesT[:, ch * 128:(ch + 1) * 128],
                                    identity[:])
                csm = ema_sb.tile([128, 128], F32, tag="csm")
                nc.vector.tensor_copy(csm[:], pst[:])
                nc.sync.dma_start(out=rawc_d[ch * 128:(ch + 1) * 128, :], in_=csm[:])
                for c in range(4):
                    nc.tensor.matmul(am[c][:], lhsT=oh_e[:, c * 128:(c + 1) * 128],
                                     rhs=csm[:], start=(ch == 0), stop=(ch == 63))
                nc.tensor.matmul(an[:], lhsT=ones_col[:], rhs=oh_e[:],
                                 start=(ch == 0), stop=(ch == 63))
            # blends: new = DECAY*acc + (1-DECAY)*sum
            mascale = ema_sb.tile([128, 4, CD], F32, tag="mas")
            nc.vector.tensor_scalar(mascale[:], macc_s[:], DECAY, None, OP.mult)
            for c in range(4):
                nm = ema_sb.tile([128, CD], F32, tag="nm")
                nc.vector.scalar_tensor_tensor(nm[:], am[c][:], 1.0 - DECAY,
                                               mascale[:, c, :], OP.mult, OP.add)
                nc.sync.dma_start(out=newm_d[c], in_=nm[:])
            nasc = ema_sb.tile([1, NT], F32, tag="nas")
            nc.vector.tensor_scalar(nasc[:], nacc_s[:], DECAY, None, OP.mult)
            nn_t = ema_sb.tile([1, NT], F32, tag="nn")
            nc.vector.scalar_tensor_tensor(nn_t[:], an[:], 1.0 - DECAY,
                                           nasc[:], OP.mult, OP.add)
            nc.sync.dma_start(out=newn_d[:], in_=nn_t[:])

        ctx.close()

    nc.compile()
    return nc


_cached = None


def _get_program():
    global _cached
    if _cached is None:
        _cached = build_program()
    return _cached


def make_in_maps(x, emb, W_i, W_s, W_h, W_t, W_n, W_code, W_out, b_out,
                 codebook, m_acc, n_acc):
    x_sm = x.T.reshape(-1).astype(np.int64)            # sample-major tokens
    et = np.ascontiguousarray(emb[x_sm].T)             # (256, 4096)
    xmask = (x != 0).astype(np.float32)
    cb_ext = np.vstack([np.zeros((1, CD), np.float32), codebook])
    cw = cb_ext @ W_s                                   # (513, 256)
    cw_pad = np.zeros((CWC * 128, D), np.float32)
    cw_pad[:NT + 1] = cw
    cw_pack = np.ascontiguousarray(
        cw_pad.reshape(CWC, 128, D).transpose(1, 0, 2).reshape(128, CWC * D))
    cbt = np.ascontiguousarray(codebook.T)
    macc_pack = np.ascontiguousarray(
        m_acc.reshape(4, 128, CD).transpose(1, 0, 2).reshape(128, 4 * CD))
    iota130 = np.broadcast_to(np.arange(DEPTH, dtype=np.float32), (B, DEPTH)).copy()
    i640 = (np.arange(128, dtype=np.float32)[:, None]
            + 128.0 * np.arange(CWC, dtype=np.float32)[None, :]).copy()
    iota512 = np.broadcast_to(np.arange(NT, dtype=np.float32), (128, NT)).copy()

    common = dict(
        et=et, xmask=xmask, wi=W_i, wh=W_h, wt=W_t, wn=W_n, wcode=W_code,
        cw=cw_pack, cbt=cbt, macc=macc_pack,
        nacc=np.ascontiguousarray(n_acc.reshape(1, NT)),
        iota130=iota130, i640=i640, iota512=iota512,
    )
    in_maps = []
    for core in range(NCORES):
        sl = slice(core * VS, (core + 1) * VS)
        wout_pack = np.ascontiguousarray(
            W_out[:, sl].reshape(KC, 128, VS).transpose(1, 0, 2).reshape(128, KC * VS))
        bout_rep = np.broadcast_to(b_out[sl], (128, VS)).copy()
        in_maps.append(dict(common, wout=wout_pack, bout=bout_rep))
    return in_maps


def assemble_outputs(results):
    logits_sm = np.concatenate([results[i]["logits"] for i in range(NCORES)], axis=1)
    tlogits = logits_sm.reshape(L, B, V).transpose(1, 0, 2)
    r0 = results[0]
    pushes = r0["pushes"].reshape(2, B, L).transpose(1, 2, 0).astype(np.int32)
    raw_codes = r0["rawc"].reshape(L, 2, B, CD).transpose(2, 0, 1, 3)
    new_m = r0["newm"].reshape(NT, CD)
    new_n = r0["newn"].reshape(NT)
    return tlogits, pushes, raw_codes, new_m, new_n


def kernel(x, emb, W_i, W_s, W_h, W_t, W_n, W_code, W_out, b_out,
           codebook, m_acc, n_acc, _spmd_kwargs=None):
    args = [np.asarray(x)] + [
        np.asarray(a, np.float32)
        for a in (emb, W_i, W_s, W_h, W_t, W_n, W_code, W_out, b_out,
                  codebook, m_acc, n_acc)]
    in_maps = make_in_maps(*args)
    nc = _get_program()
    kw = _spmd_kwargs or {}
    res = run_bass_kernel_spmd(nc, in_maps, core_ids=list(range(NCORES)), **kw)
    out = assemble_outputs(res.results)
    kernel.last_run = res
    return out
